# revision 20
# baseline (speedup 1.0000x reference)
"""ColBERT scoring kernel for Trainium2 (Bass/Tile), data-parallel over batch.

Reference computation (per batch b):
    Q = l2norm(q_hidden[b] @ W)                     # [LQ, DIM]
    D = l2norm((d_hidden[b] * mask[b,:,None]) @ W)  # [LD, DIM]
    score[b] = sum_q max_k (Q @ D.T)[q, k]

Sharding: batch dim B=64 split over 8 NeuronCores (8 batches/core), W replicated.

Device strategy (v2, fp8):
  - All PE inputs stream as fp8_e4m3 (half the HBM bytes of fp16). W is
    pre-scaled x32 on the host so its values sit in e4m3's normal range;
    the L2 normalization makes every score invariant to that scale.
  - Projections run as DoubleRow fp8 matmuls: contraction 2x128=256 per
    pass, 2x the MACs/cycle of fp16. Host layout is chunk-major
    [P, HC, cols] so a [:, 2c:2c+2, :] slice is exactly the two k-planes
    DoubleRow wants.
  - Inputs are packed into 5 dram tensors = 5 DMA descriptor issues
    ([w|dt0|dt1], q, [dt2|dt3], [dt4|dt5], [dt6|dt7]), alternated across
    both hardware DGEs (sync + scalar) so descriptor enqueues overlap.
  - Per batch: pd = W'^T d'^T (PSUM) -> cpd fp16 SBUF copy (ACT, one
    PSUM read) -> sqd = cpd*cpd (GpSimd, SBUF-only) -> column sums
    broadcast via ones-matmul (PE) -> r = rsqrt(ss+eps^2) (ACT) ->
    psim = qT^T cpd (PE, unnormalized) -> fused DVE tensor_tensor_reduce:
    max_k(psim * r, init=0) (the init-0 doubles as the empty-column clamp)
    -> * 1/||q|| (applied after the max; it commutes).
  - Doc tokens are host-compacted to K_CAP=320 columns (5.7 sigma above
    the Binomial(512,1/2) mean); overflow or non-0/1 masks fall back to
    an exact full-width build.

COLBERT_MM_MODE: f8 (default) | f16 (fp16 inputs, normal-rate matmuls)
"""

import os

import numpy as np

B, LQ, LD, H, DIM = 64, 128, 512, 768, 128
NCORES = 8
BLOC = B // NCORES  # 8 batches per core
P = 128
HC = H // P  # 6 contraction chunks of 128
NQ = BLOC * LQ  # 1024
# norm eps: real sumsq values are ~4e4 (W pre-scaled x32), so 1e-8 is
# negligible there, while zero (padded/masked) columns get r = 1e4 — finite
# in fp16, and 0 * 1e4 = 0 keeps their sims at exactly 0 like the reference.
EPS2 = 1e-8
W_SCALE = 32.0  # host pre-scale for W -> e4m3 normal range; normalization
                # cancels it exactly

MM_MODE = os.environ.get("COLBERT_MM_MODE", "f8")

_cache = {}


def _build(kd=320):
    import concourse.bass as bass
    import concourse.tile as tile
    from concourse import bacc, mybir

    f32 = mybir.dt.float32
    f16 = mybir.dt.float16
    fp8 = mybir.dt.float8e4
    use_f8 = MM_MODE == "f8"
    in_dt = fp8 if use_f8 else f16
    DR = mybir.MatmulPerfMode.DoubleRow if use_f8 else None
    # chunk granularity of the projection contraction
    NCH = HC // 2 if use_f8 else HC  # 3 DoubleRow passes or 6 plain
    CP = 2 if use_f8 else 1  # k-planes per pass

    nc = bacc.Bacc("TRN2", target_bir_lowering=False, debug=False,
                   num_devices=NCORES)

    WCOLS = HC * DIM   # 768 w columns, chunk-major
    DCOLS = HC * kd    # per-batch dt columns, chunk-major
    # five input streams; [w|dt0|dt1] packed so one descriptor issue covers
    # the whole kernel prologue
    wd01 = nc.dram_tensor("wd01", [P, WCOLS + 2 * DCOLS], in_dt,
                          kind="ExternalInput").ap()
    qt = nc.dram_tensor("qt", [P, HC * NQ], in_dt, kind="ExternalInput").ap()
    dt23 = nc.dram_tensor("dt23", [P, 2 * DCOLS], in_dt,
                          kind="ExternalInput").ap()
    dt45 = nc.dram_tensor("dt45", [P, 2 * DCOLS], in_dt,
                          kind="ExternalInput").ap()
    dt67 = nc.dram_tensor("dt67", [P, 2 * DCOLS], in_dt,
                          kind="ExternalInput").ap()
    out = nc.dram_tensor("scores", [BLOC, 1], f32, kind="ExternalOutput").ap()

    with tile.TileContext(nc) as tc:
        with (
            tc.tile_pool(name="const", bufs=1) as const,
            tc.tile_pool(name="work", bufs=3) as work,
            tc.tile_pool(name="ps_d", bufs=4, space="PSUM") as ps_d,
            tc.tile_pool(name="ps_q", bufs=1, space="PSUM") as ps_q,
            tc.tile_pool(name="ps_nrm", bufs=1, space="PSUM") as ps_nrm,
            tc.tile_pool(name="ps_sim", bufs=2, space="PSUM") as ps_sim,
        ):
            # ---- input DMAs: 5 issues, alternating across the two HWDGEs ----
            wd01_sb = const.tile([P, WCOLS + 2 * DCOLS], in_dt)
            q_sb = const.tile([P, HC, NQ], in_dt)
            dt23_sb = const.tile([P, 2, HC, kd], in_dt)
            dt45_sb = const.tile([P, 2, HC, kd], in_dt)
            dt67_sb = const.tile([P, 2, HC, kd], in_dt)
            # all on the SP DGE: the scalar engine's queue must stay free of
            # descriptor issues so its ACT table loads don't delay the stream
            nc.sync.dma_start(out=wd01_sb, in_=wd01)
            nc.sync.dma_start(out=q_sb, in_=qt)
            nc.sync.dma_start(out=dt23_sb, in_=dt23)
            nc.sync.dma_start(out=dt45_sb, in_=dt45)
            nc.sync.dma_start(out=dt67_sb, in_=dt67)

            # chunk-major views of the packed prologue tensor
            w_v = wd01_sb[:, 0:WCOLS].rearrange("p (c d) -> p c d", c=HC)
            d0_v = wd01_sb[:, WCOLS:WCOLS + DCOLS].rearrange(
                "p (c k) -> p c k", c=HC)
            d1_v = wd01_sb[:, WCOLS + DCOLS:WCOLS + 2 * DCOLS].rearrange(
                "p (c k) -> p c k", c=HC)

            def dview(b):
                if b == 0:
                    return d0_v
                if b == 1:
                    return d1_v
                t = (dt23_sb, dt45_sb, dt67_sb)[(b - 2) // 2]
                return t[:, b % 2]

            # ---- constants ----
            ones_pk = const.tile([P, P], f16)
            nc.gpsimd.memset(ones_pk, 1.0)
            ones_c1 = const.tile([P, 1], f16)
            nc.gpsimd.memset(ones_c1, 1.0)
            ones_f32 = const.tile([P, 1], f32)
            nc.gpsimd.memset(ones_f32, 1.0)
            eps_c = const.tile([P, 1], f32)
            nc.gpsimd.memset(eps_c, EPS2)

            qT_all = const.tile([P, NQ], f16)     # unnormalized Q^T, all b
            sqq = const.tile([P, NQ], f16)        # qT^2 (gpsimd)
            rq = const.tile([P, BLOC], f32)       # 1/||q|| per (LQ, b)
            scores_cols = const.tile([P, BLOC], f32)

            # dummy rsqrt on a 1-elem tile: forces walrus's one-and-only
            # ACT table load to be the abs_reciprocal_sqrt set (which also
            # contains Copy/Square), hidden under the initial DMA wait
            dummy = const.tile([P, 1], f32)
            nc.gpsimd.memset(dummy, 1.0)
            nc.scalar.activation(
                dummy, dummy,
                mybir.ActivationFunctionType.Abs_reciprocal_sqrt,
                bias=eps_c)

            pssq = ps_sim.tile([P, BLOC], f32, tag="psim", name="pssq")

            def wslice(c):
                # stationary weight chunk: [128, CP, 128]
                return w_v[:, CP * c:CP * (c + 1), :]

            def proj_pair(b0, pds):
                # two batches interleaved per weight chunk to share LDWEIGHTS
                for c in range(NCH):
                    wc = wslice(c)
                    for i, pd in enumerate(pds):
                        dv = dview(b0 + i)
                        nc.tensor.matmul(
                            pd,
                            wc,
                            dv[:, CP * c:CP * (c + 1), :],
                            start=(c == 0), stop=(c == NCH - 1),
                            perf_mode=DR,
                        )

            def q_proj_g(g):
                psq = ps_q.tile([P, 512], f32, name=f"psq{g}", tag="pq")
                for c in range(NCH):
                    nc.tensor.matmul(
                        psq,
                        wslice(c),
                        q_sb[:, CP * c:CP * (c + 1), g * 512:(g + 1) * 512],
                        start=(c == 0), stop=(c == NCH - 1),
                        perf_mode=DR,
                    )
                qs = qT_all[:, g * 512:(g + 1) * 512]
                nc.vector.tensor_copy(qs, psq)
                nc.gpsimd.tensor_mul(sqq[:, g * 512:(g + 1) * 512], qs, qs)
                for b in range(4 * g, 4 * g + 4):
                    nc.tensor.matmul(
                        pssq[:, b:b + 1],
                        sqq[:, b * LQ:(b + 1) * LQ],
                        ones_c1,
                        start=True, stop=True,
                    )
                if g == 1:
                    nc.scalar.activation(
                        rq, pssq,
                        mybir.ActivationFunctionType.Abs_reciprocal_sqrt,
                        bias=eps_c)

            def post_stage(b, pd):
                # one PSUM read of pd: fp16 copy (ACT) that feeds both the
                # norm square and the sim matmul; all further elementwise is
                # 16-bit SBUF->SBUF (DVE 2x rate / gpsimd-legal)
                cpd = work.tile([P, kd], f16, tag="cpd", name=f"cpd{b}",
                                bufs=3)
                nc.scalar.copy(cpd, pd)
                sqd = work.tile([P, kd], f16, tag="sqd", name=f"sqd{b}",
                                bufs=3)
                if b in (3, 7):
                    nc.vector.tensor_mul(sqd, cpd, cpd)
                else:
                    nc.gpsimd.tensor_mul(sqd, cpd, cpd)
                pssb = ps_nrm.tile([P, kd], f32, name=f"pssb{b}", tag="pssb")
                nc.tensor.matmul(pssb, ones_pk, sqd, start=True, stop=True)
                r = work.tile([P, kd], f16, tag="r", name=f"r{b}", bufs=3)
                nc.scalar.activation(
                    r, pssb,
                    mybir.ActivationFunctionType.Abs_reciprocal_sqrt,
                    bias=eps_c)
                # normalize before the sim matmul; the scale commutes
                # through the gemm
                dhat = work.tile([P, kd], f16, tag="dhat", name=f"dhat{b}",
                                 bufs=3)
                nc.vector.tensor_mul(dhat, cpd, r)
                psim = ps_sim.tile([P, kd], f32, name=f"psim{b}", tag="psim")
                nc.tensor.matmul(
                    psim,
                    qT_all[:, b * LQ:(b + 1) * LQ],
                    dhat,
                    start=True, stop=True,
                )
                nc.vector.reduce_max(scores_cols[:, b:b + 1], psim,
                                     axis=mybir.AxisListType.X)

            # ---- schedule: pair(0,1) as soon as wd01 lands; q-proj groups
            # next; then each pair's PROJECTIONS are emitted one pair AHEAD
            # of its post stages, so the PE never waits on the elementwise
            # chain of the pair it just projected
            pds = {}
            for b in (0, 1):
                pds[b] = ps_d.tile([P, kd], f32, name=f"pd{b}", tag="pd")
            proj_pair(0, (pds[0], pds[1]))
            q_proj_g(0)
            q_proj_g(1)
            for pair in range(1, 4):
                b0 = 2 * pair
                pds[b0] = ps_d.tile([P, kd], f32, name=f"pd{b0}", tag="pd")
                pds[b0 + 1] = ps_d.tile([P, kd], f32, name=f"pd{b0 + 1}",
                                        tag="pd")
                proj_pair(b0, (pds[b0], pds[b0 + 1]))
                post_stage(b0 - 2, pds[b0 - 2])
                post_stage(b0 - 1, pds[b0 - 1])
            post_stage(4, pds[4])
            post_stage(5, pds[5])
            post_stage(6, pds[6])
            post_stage(7, pds[7])

            # ---- final: clamp at 0 (reference's masked columns always
            # contribute a 0 sim), apply 1/||q|| (commutes with the max),
            # sum over LQ
            nc.vector.tensor_scalar_max(scores_cols, scores_cols, 0.0)
            nc.vector.tensor_mul(scores_cols, scores_cols, rq)
            pfin = ps_sim.tile([BLOC, 1], f32, tag="psim", name="pfin")
            nc.tensor.matmul(pfin, scores_cols, ones_f32, start=True, stop=True)
            scores_sb = work.tile([BLOC, 1], f32, tag="fin", bufs=1)
            nc.vector.tensor_copy(scores_sb, pfin)
            nc.sync.dma_start(out=out, in_=scores_sb)

    nc.compile()
    return nc


def _cvt(a):
    if MM_MODE == "f8":
        import ml_dtypes
        return np.ascontiguousarray(a.astype(ml_dtypes.float8_e4m3))
    return np.ascontiguousarray(a.astype(np.float16))


def kernel(q_hidden, d_hidden, W, doc_mask):
    from concourse.bass_utils import run_bass_kernel_spmd

    q_hidden = np.asarray(q_hidden, dtype=np.float32)
    d_hidden = np.asarray(d_hidden, dtype=np.float32)
    W = np.asarray(W, dtype=np.float32)
    doc_mask = np.asarray(doc_mask)

    # fixed compaction capacity (5.7 sigma over the Binomial(512,1/2) mean);
    # overflow or a non-0/1 mask falls back to the exact full-width build
    K_CAP = 320
    counts = (doc_mask != 0).sum(axis=1)
    compact = counts.max() <= K_CAP and bool(np.isin(doc_mask, (0, 1)).all())
    kd = K_CAP if compact else LD
    key = ("nc", kd)
    if key not in _cache:
        _cache[key] = _build(kd=kd)
    nc = _cache[key]

    if compact:
        # keep only unmasked doc tokens; zero-pad to K_CAP. Padded columns
        # reach the score only through max(.,0), which matches the
        # reference's masked columns exactly.
        d_m = np.zeros((B, kd, H), dtype=np.float32)
        mask_b = doc_mask != 0
        for b in range(B):
            sel = d_hidden[b][mask_b[b]]
            d_m[b, :len(sel)] = sel
    else:
        d_m = d_hidden * doc_mask[..., None].astype(np.float32)

    # W pre-scale: e4m3's normal range starts at 2^-6; W~N(0,0.02^2) would
    # land subnormal. L2 normalization cancels the scale exactly.
    wt = _cvt((W * W_SCALE).reshape(HC, P, DIM)
              .transpose(1, 0, 2).reshape(P, HC * DIM))
    in_maps = []
    for c in range(NCORES):
        sl = slice(c * BLOC, (c + 1) * BLOC)
        # qt[p, cc*NQ + b*LQ + l] = q[b, l, cc*P + p]
        qtc = _cvt(q_hidden[sl].reshape(BLOC, LQ, HC, P)
                   .transpose(3, 2, 0, 1).reshape(P, HC * NQ))
        # per-batch chunk-major dt[b][p, cc*kd + l] = d_m[b, l, cc*P + p]
        dtc = _cvt(d_m[sl].reshape(BLOC, kd, HC, P)
                   .transpose(0, 3, 2, 1).reshape(BLOC, P, HC * kd))
        wd01 = np.concatenate([wt, dtc[0], dtc[1]], axis=1)
        in_maps.append({
            "wd01": np.ascontiguousarray(wd01),
            "qt": qtc,
            "dt23": np.ascontiguousarray(
                dtc[2:4].transpose(1, 0, 2).reshape(P, -1)),
            "dt45": np.ascontiguousarray(
                dtc[4:6].transpose(1, 0, 2).reshape(P, -1)),
            "dt67": np.ascontiguousarray(
                dtc[6:8].transpose(1, 0, 2).reshape(P, -1)),
        })

    trace = os.environ.get("COLBERT_TRACE", "0") == "1"
    res = run_bass_kernel_spmd(nc, in_maps, core_ids=list(range(NCORES)),
                               trace=trace)
    _cache["last_results"] = res
    return np.concatenate([r["scores"].reshape(BLOC) for r in res.results])


# revision 27
# speedup vs baseline: 1.0933x; 1.0933x over previous
"""ColBERT scoring kernel for Trainium2 (Bass/Tile), data-parallel over batch.

Reference computation (per batch b):
    Q = l2norm(q_hidden[b] @ W)                     # [LQ, DIM]
    D = l2norm((d_hidden[b] * mask[b,:,None]) @ W)  # [LD, DIM]
    score[b] = sum_q max_k (Q @ D.T)[q, k]

Sharding: batch dim B=64 split over 8 NeuronCores (8 batches/core), W replicated.

Device strategy (v2, fp8):
  - All PE inputs stream as fp8_e4m3 (half the HBM bytes of fp16). W is
    pre-scaled x32 on the host so its values sit in e4m3's normal range;
    the L2 normalization makes every score invariant to that scale.
  - Projections run as DoubleRow fp8 matmuls: contraction 2x128=256 per
    pass, 2x the MACs/cycle of fp16. Host layout is chunk-major
    [P, HC, cols] so a [:, 2c:2c+2, :] slice is exactly the two k-planes
    DoubleRow wants.
  - Inputs are packed into 5 dram tensors = 5 DMA descriptor issues
    ([w|dt0|dt1], q, [dt2|dt3], [dt4|dt5], [dt6|dt7]), alternated across
    both hardware DGEs (sync + scalar) so descriptor enqueues overlap.
  - Per batch: pd = W'^T d'^T (PSUM) -> cpd fp16 SBUF copy (ACT, one
    PSUM read) -> sqd = cpd*cpd (GpSimd, SBUF-only) -> column sums
    broadcast via ones-matmul (PE) -> r = rsqrt(ss+eps^2) (ACT) ->
    psim = qT^T cpd (PE, unnormalized) -> fused DVE tensor_tensor_reduce:
    max_k(psim * r, init=0) (the init-0 doubles as the empty-column clamp)
    -> * 1/||q|| (applied after the max; it commutes).
  - Doc tokens are host-compacted to K_CAP=320 columns (5.7 sigma above
    the Binomial(512,1/2) mean); overflow or non-0/1 masks fall back to
    an exact full-width build.

COLBERT_MM_MODE: f8 (default) | f16 (fp16 inputs, normal-rate matmuls)
"""

import os

import numpy as np

B, LQ, LD, H, DIM = 64, 128, 512, 768, 128
NCORES = 8
BLOC = B // NCORES  # 8 batches per core
P = 128
HC = H // P  # 6 contraction chunks of 128
NQ = BLOC * LQ  # 1024
# norm eps: real sumsq values are ~4e4 (W pre-scaled x32), so 1e-8 is
# negligible there, while zero (padded/masked) columns get r = 1e4 — finite
# in fp16, and 0 * 1e4 = 0 keeps their sims at exactly 0 like the reference.
EPS2 = 1e-8
W_SCALE = 32.0  # host pre-scale for W -> e4m3 normal range; normalization
                # cancels it exactly

MM_MODE = os.environ.get("COLBERT_MM_MODE", "f8")

_cache = {}


def _build(kd=320):
    import concourse.bass as bass
    import concourse.tile as tile
    from concourse import bacc, mybir

    f32 = mybir.dt.float32
    f16 = mybir.dt.float16
    fp8 = mybir.dt.float8e4
    use_f8 = MM_MODE == "f8"
    in_dt = fp8 if use_f8 else f16
    DR = mybir.MatmulPerfMode.DoubleRow if use_f8 else None
    # chunk granularity of the projection contraction
    NCH = HC // 2 if use_f8 else HC  # 3 DoubleRow passes or 6 plain
    CP = 2 if use_f8 else 1  # k-planes per pass

    nc = bacc.Bacc("TRN2", target_bir_lowering=False, debug=False,
                   num_devices=NCORES)

    WCOLS = HC * DIM   # 768 w columns, chunk-major
    DCOLS = HC * kd    # per-batch dt columns, chunk-major
    # five input streams; [w|dt0|dt1] packed so one descriptor issue covers
    # the whole kernel prologue
    wd01 = nc.dram_tensor("wd01", [P, WCOLS + 2 * DCOLS], in_dt,
                          kind="ExternalInput").ap()
    qta = nc.dram_tensor("qta", [P, HC * 512], in_dt,
                         kind="ExternalInput").ap()
    qtb = nc.dram_tensor("qtb", [P, HC * 512], in_dt,
                         kind="ExternalInput").ap()
    dt23 = nc.dram_tensor("dt23", [P, 2 * DCOLS], in_dt,
                          kind="ExternalInput").ap()
    dt45 = nc.dram_tensor("dt45", [P, 2 * DCOLS], in_dt,
                          kind="ExternalInput").ap()
    dt67 = nc.dram_tensor("dt67", [P, 2 * DCOLS], in_dt,
                          kind="ExternalInput").ap()
    out = nc.dram_tensor("scores", [BLOC, 1], f32, kind="ExternalOutput").ap()

    with tile.TileContext(nc) as tc:
        with (
            tc.tile_pool(name="const", bufs=1) as const,
            tc.tile_pool(name="work", bufs=3) as work,
            tc.tile_pool(name="ps_d", bufs=3, space="PSUM") as ps_d,
            tc.tile_pool(name="ps_q", bufs=1, space="PSUM") as ps_q,
            tc.tile_pool(name="ps_nrm", bufs=2, space="PSUM") as ps_nrm,
            tc.tile_pool(name="ps_sim", bufs=2, space="PSUM") as ps_sim,
        ):
            # ---- input DMAs, all on the SP DGE (the scalar engine's queue
            # must stay free of descriptor issues so its ACT table loads
            # don't delay the stream). Stream order = consumption order;
            # q ships as two halves so q-proj g0 can fill the PE gap right
            # after the first projection pair.
            wd01_sb = const.tile([P, WCOLS + 2 * DCOLS], in_dt)
            qa_sb = const.tile([P, HC, 512], in_dt)
            qb_sb = const.tile([P, HC, 512], in_dt)
            dt23_sb = const.tile([P, 2, HC, kd], in_dt)
            dt45_sb = const.tile([P, 2, HC, kd], in_dt)
            dt67_sb = const.tile([P, 2, HC, kd], in_dt)
            nc.sync.dma_start(out=wd01_sb, in_=wd01)
            nc.sync.dma_start(out=qa_sb, in_=qta)
            nc.sync.dma_start(out=dt23_sb, in_=dt23)
            nc.sync.dma_start(out=qb_sb, in_=qtb)
            nc.sync.dma_start(out=dt45_sb, in_=dt45)
            nc.sync.dma_start(out=dt67_sb, in_=dt67)

            # chunk-major views of the packed prologue tensor
            w_v = wd01_sb[:, 0:WCOLS].rearrange("p (c d) -> p c d", c=HC)
            d0_v = wd01_sb[:, WCOLS:WCOLS + DCOLS].rearrange(
                "p (c k) -> p c k", c=HC)
            d1_v = wd01_sb[:, WCOLS + DCOLS:WCOLS + 2 * DCOLS].rearrange(
                "p (c k) -> p c k", c=HC)

            def dview(b):
                if b == 0:
                    return d0_v
                if b == 1:
                    return d1_v
                t = (dt23_sb, dt45_sb, dt67_sb)[(b - 2) // 2]
                return t[:, b % 2]

            # ---- constants ----
            ones_pk = const.tile([P, P], f16)
            nc.gpsimd.memset(ones_pk, 1.0)
            ones_c1 = const.tile([P, 1], f16)
            nc.gpsimd.memset(ones_c1, 1.0)
            ones_f32 = const.tile([P, 1], f32)
            nc.gpsimd.memset(ones_f32, 1.0)
            eps_c = const.tile([P, 1], f32)
            nc.gpsimd.memset(eps_c, EPS2)

            qT_all = const.tile([P, NQ], f16)     # unnormalized Q^T, all b
            sqq = const.tile([P, NQ], f16)        # qT^2 (gpsimd)
            rq = const.tile([P, BLOC], f32)       # 1/||q|| per (LQ, b)
            scores_cols = const.tile([P, BLOC], f32)

            # dummy rsqrt on a 1-elem tile: forces walrus's one-and-only
            # ACT table load to be the abs_reciprocal_sqrt set (which also
            # contains Copy/Square), hidden under the initial DMA wait
            dummy = const.tile([P, 1], f32)
            nc.gpsimd.memset(dummy, 1.0)
            nc.scalar.activation(
                dummy, dummy,
                mybir.ActivationFunctionType.Abs_reciprocal_sqrt,
                bias=eps_c)

            pssq = ps_sim.tile([P, BLOC], f32, tag="psim", name="pssq")

            def wslice(c):
                # stationary weight chunk: [128, CP, 128]
                return w_v[:, CP * c:CP * (c + 1), :]

            def proj_pair(b0, pds):
                # two batches interleaved per weight chunk to share LDWEIGHTS
                for c in range(NCH):
                    wc = wslice(c)
                    for i, pd in enumerate(pds):
                        dv = dview(b0 + i)
                        nc.tensor.matmul(
                            pd,
                            wc,
                            dv[:, CP * c:CP * (c + 1), :],
                            start=(c == 0), stop=(c == NCH - 1),
                            perf_mode=DR,
                        )

            def q_proj_g(g):
                qg_sb = qa_sb if g == 0 else qb_sb
                psq = ps_q.tile([P, 512], f32, name=f"psq{g}", tag="pq")
                for c in range(NCH):
                    nc.tensor.matmul(
                        psq,
                        wslice(c),
                        qg_sb[:, CP * c:CP * (c + 1), :],
                        start=(c == 0), stop=(c == NCH - 1),
                        perf_mode=DR,
                    )
                qs = qT_all[:, g * 512:(g + 1) * 512]
                nc.vector.tensor_copy(qs, psq)
                nc.gpsimd.tensor_mul(sqq[:, g * 512:(g + 1) * 512], qs, qs)
                for b in range(4 * g, 4 * g + 4):
                    nc.tensor.matmul(
                        pssq[:, b:b + 1],
                        sqq[:, b * LQ:(b + 1) * LQ],
                        ones_c1,
                        start=True, stop=True,
                    )
                if g == 1:
                    nc.scalar.activation(
                        rq, pssq,
                        mybir.ActivationFunctionType.Abs_reciprocal_sqrt,
                        bias=eps_c)

            def post_stage(b, pd):
                # one PSUM read of pd: fp16 copy (ACT) that feeds both the
                # norm square and the sim matmul; all further elementwise is
                # 16-bit SBUF->SBUF (DVE 2x rate / gpsimd-legal)
                cpd = work.tile([P, kd], f16, tag="cpd", name=f"cpd{b}",
                                bufs=3)
                nc.scalar.copy(cpd, pd)
                sqd = work.tile([P, kd], f16, tag="sqd", name=f"sqd{b}",
                                bufs=3)
                if b in (3, 7):
                    nc.vector.tensor_mul(sqd, cpd, cpd)
                else:
                    nc.gpsimd.tensor_mul(sqd, cpd, cpd)
                pssb = ps_nrm.tile([P, kd], f32, name=f"pssb{b}", tag="pssb")
                nc.tensor.matmul(pssb, ones_pk, sqd, start=True, stop=True)
                r = work.tile([P, kd], f16, tag="r", name=f"r{b}", bufs=3)
                nc.scalar.activation(
                    r, pssb,
                    mybir.ActivationFunctionType.Abs_reciprocal_sqrt,
                    bias=eps_c)
                # normalize before the sim matmul; the scale commutes
                # through the gemm
                dhat = work.tile([P, kd], f16, tag="dhat", name=f"dhat{b}",
                                 bufs=3)
                nc.vector.tensor_mul(dhat, cpd, r)
                psim = ps_sim.tile([P, kd], f32, name=f"psim{b}", tag="psim")
                nc.tensor.matmul(
                    psim,
                    qT_all[:, b * LQ:(b + 1) * LQ],
                    dhat,
                    start=True, stop=True,
                )
                nc.vector.reduce_max(scores_cols[:, b:b + 1], psim,
                                     axis=mybir.AxisListType.X)

            # ---- schedule: pair(0,1) as soon as wd01 lands; q-proj groups
            # fill the PE while pair01's elementwise chain runs; then pairs
            # with their posts immediately after
            pd0 = ps_d.tile([P, kd], f32, name="pd0", tag="pd")
            pd1 = ps_d.tile([P, kd], f32, name="pd1", tag="pd")
            proj_pair(0, (pd0, pd1))
            q_proj_g(0)
            q_proj_g(1)
            post_stage(0, pd0)
            post_stage(1, pd1)
            for pair in range(1, 4):
                b0 = 2 * pair
                pda = ps_d.tile([P, kd], f32, name=f"pd{b0}", tag="pd")
                pdb = ps_d.tile([P, kd], f32, name=f"pd{b0 + 1}", tag="pd")
                proj_pair(b0, (pda, pdb))
                post_stage(b0, pda)
                post_stage(b0 + 1, pdb)

            # ---- final: clamp at 0 (reference's masked columns always
            # contribute a 0 sim), apply 1/||q|| (commutes with the max),
            # sum over LQ
            nc.vector.tensor_scalar_max(scores_cols, scores_cols, 0.0)
            nc.vector.tensor_mul(scores_cols, scores_cols, rq)
            pfin = ps_sim.tile([BLOC, 1], f32, tag="psim", name="pfin")
            nc.tensor.matmul(pfin, scores_cols, ones_f32, start=True, stop=True)
            scores_sb = work.tile([BLOC, 1], f32, tag="fin", bufs=1)
            nc.vector.tensor_copy(scores_sb, pfin)
            nc.sync.dma_start(out=out, in_=scores_sb)

    nc.compile()
    return nc


def _cvt(a):
    if MM_MODE == "f8":
        import ml_dtypes
        return np.ascontiguousarray(a.astype(ml_dtypes.float8_e4m3))
    return np.ascontiguousarray(a.astype(np.float16))


def kernel(q_hidden, d_hidden, W, doc_mask):
    from concourse.bass_utils import run_bass_kernel_spmd

    q_hidden = np.asarray(q_hidden, dtype=np.float32)
    d_hidden = np.asarray(d_hidden, dtype=np.float32)
    W = np.asarray(W, dtype=np.float32)
    doc_mask = np.asarray(doc_mask)

    # fixed compaction capacity (5.7 sigma over the Binomial(512,1/2) mean);
    # overflow or a non-0/1 mask falls back to the exact full-width build
    K_CAP = 320
    counts = (doc_mask != 0).sum(axis=1)
    compact = counts.max() <= K_CAP and bool(np.isin(doc_mask, (0, 1)).all())
    kd = K_CAP if compact else LD
    key = ("nc", kd)
    if key not in _cache:
        _cache[key] = _build(kd=kd)
    nc = _cache[key]

    if compact:
        # keep only unmasked doc tokens; zero-pad to K_CAP. Padded columns
        # reach the score only through max(.,0), which matches the
        # reference's masked columns exactly.
        d_m = np.zeros((B, kd, H), dtype=np.float32)
        mask_b = doc_mask != 0
        for b in range(B):
            sel = d_hidden[b][mask_b[b]]
            d_m[b, :len(sel)] = sel
    else:
        d_m = d_hidden * doc_mask[..., None].astype(np.float32)

    # W pre-scale: e4m3's normal range starts at 2^-6; W~N(0,0.02^2) would
    # land subnormal. L2 normalization cancels the scale exactly.
    wt = _cvt((W * W_SCALE).reshape(HC, P, DIM)
              .transpose(1, 0, 2).reshape(P, HC * DIM))
    in_maps = []
    for c in range(NCORES):
        sl = slice(c * BLOC, (c + 1) * BLOC)
        # q[p, cc, b*LQ + l] = q[b, l, cc*P + p], split into token halves
        qtc = _cvt(q_hidden[sl].reshape(BLOC, LQ, HC, P)
                   .transpose(3, 2, 0, 1).reshape(P, HC, NQ))
        # per-batch chunk-major dt[b][p, cc*kd + l] = d_m[b, l, cc*P + p]
        dtc = _cvt(d_m[sl].reshape(BLOC, kd, HC, P)
                   .transpose(0, 3, 2, 1).reshape(BLOC, P, HC * kd))
        wd01 = np.concatenate([wt, dtc[0], dtc[1]], axis=1)
        in_maps.append({
            "wd01": np.ascontiguousarray(wd01),
            "qta": np.ascontiguousarray(qtc[:, :, :512].reshape(P, -1)),
            "qtb": np.ascontiguousarray(qtc[:, :, 512:].reshape(P, -1)),
            "dt23": np.ascontiguousarray(
                dtc[2:4].transpose(1, 0, 2).reshape(P, -1)),
            "dt45": np.ascontiguousarray(
                dtc[4:6].transpose(1, 0, 2).reshape(P, -1)),
            "dt67": np.ascontiguousarray(
                dtc[6:8].transpose(1, 0, 2).reshape(P, -1)),
        })

    trace = os.environ.get("COLBERT_TRACE", "0") == "1"
    res = run_bass_kernel_spmd(nc, in_maps, core_ids=list(range(NCORES)),
                               trace=trace)
    _cache["last_results"] = res
    return np.concatenate([r["scores"].reshape(BLOC) for r in res.results])
